# revision 43
# baseline (speedup 1.0000x reference)
"""Trainium2 Bass kernel for the CKTGNN batch-of-DAGs encoder.

Strategy (pure data parallel, B=4096 split over 8 NeuronCores, 512 graphs/core):
  - one-hot inputs are built on device; per-vertex x-side GRU contributions
    come from K=109 combined one-hot+h-tail matmuls (wAcomb) with biases
    folded into the one-hot columns,
  - the 12-step vertex scan keeps the GRU state batch-major; PE transposes
    (fp16) produce the feature-major copies the matmuls need,
  - the masked predecessor sum (h_in) is a per-batch-scalar axpy chain over
    previously computed gated blocks (DVE scalar_tensor_tensor, fp16 2x),
  - SBUF-only elementwise GRU ops run once per step over a chunk-blocked
    [128, 4, 304] fp16 layout (4x fewer DVE/ACT dispatches),
  - the df scatter (last-write-wins select chain) runs on GPSIMD in the
    prologue, fully overlapped with the scan.

kernel(**inputs) takes the full unsharded inputs, shards batch over the 8
cores, runs the SPMD bass kernel, and concatenates the shard outputs.
"""

from contextlib import ExitStack

import numpy as np

import concourse.bass as bass
import concourse.tile as tile
from concourse import bacc, mybir
from concourse.bass_utils import run_bass_kernel_spmd
from concourse.masks import make_identity

NCORES = 8
B = 4096
BL = B // NCORES          # batch per core
CH = BL // 128            # 128-row chunks per core
MAX_N = 12
NT = 10
PP = 9
HID = 301
HP = 304                  # padded hidden block (4B-aligned fp16 strides)
GI = 3 * HID              # 903
OUT_W = 112

f32 = mybir.dt.float32
bf16 = mybir.dt.bfloat16
f16 = mybir.dt.float16
i32 = mybir.dt.int32
OP = mybir.AluOpType
AF = mybir.ActivationFunctionType

# feature-dim k-chunks of the hidden state handled by full-K matmuls
KCH2 = [(0, 128), (128, 128)]   # tail 256:301 rides in the K=109 combo tiles

_CACHE = {}


def _body(ctx, tc, nc, d, d_out):
    cs = lambda c: slice(128 * c, 128 * (c + 1))

    consts = ctx.enter_context(tc.tile_pool(name="consts", bufs=1))
    wp = ctx.enter_context(tc.tile_pool(name="wp", bufs=1))
    big = ctx.enter_context(tc.tile_pool(name="big", bufs=1))
    pin = ctx.enter_context(tc.tile_pool(name="pin", bufs=2))
    p16 = ctx.enter_context(tc.tile_pool(name="p16", bufs=2))
    p_df = ctx.enter_context(tc.tile_pool(name="p_df", bufs=8))
    p_out = ctx.enter_context(tc.tile_pool(name="p_out", bufs=4))
    # PSUM: PR 2 x [128,1024]f32 (2 banks each) + P1 2 x [128,512]f32 +
    # PT 2 x [128,512]f16 (half-bank each)
    PR = ctx.enter_context(tc.tile_pool(name="PR", bufs=2, space="PSUM"))
    P1 = ctx.enter_context(tc.tile_pool(name="P1", bufs=2, space="PSUM"))
    PT = ctx.enter_context(tc.tile_pool(name="PT", bufs=2, space="PSUM"))

    mm = nc.tensor.matmul

    # ---------------- constants ----------------
    ident = consts.tile([128, 128], f32, name="ident", tag="ident")
    make_identity(nc, ident[:])
    ident_b = consts.tile([128, 128], bf16, name="ident_b", tag="ident_b")
    make_identity(nc, ident_b[:])
    ident_h = consts.tile([128, 128], f16, name="ident_h", tag="ident_h")
    make_identity(nc, ident_h[:])
    cmpi = consts.tile([128, NT], i32, name="cmpi", tag="cmpi")
    nc.gpsimd.iota(cmpi[:], pattern=[[1, NT]], base=0, channel_multiplier=0)
    cmpf = consts.tile([128, NT], f32, name="cmpf", tag="cmpf")
    nc.vector.tensor_copy(cmpf[:], cmpi[:])

    # ---------------- inputs first (the one-hot chain gates the scan) -----
    adj_h, feats, X19, X19b = [], [], [], []
    tfs, pfs = [], []
    for c in range(CH):
        ti = pin.tile([128, MAX_N], i32, name="ti", tag="ti")
        nc.sync.dma_start(ti[:], d["types"][cs(c), :])
        tf = pin.tile([128, MAX_N], f32, name="tf", tag="tf")
        nc.scalar.copy(tf[:], ti[:])
        tfs.append(tf)
        pi = pin.tile([128, MAX_N], i32, name="pi", tag="pi")
        nc.sync.dma_start(pi[:], d["paths"][cs(c), :])
        pf = pin.tile([128, MAX_N], f32, name="pf", tag="pf")
        nc.scalar.copy(pf[:], pi[:])
        pfs.append(pf)
        ai = pin.tile([128, MAX_N * MAX_N], i32, name="ai", tag="ai")
        nc.sync.dma_start(ai[:], d["adj"][cs(c), :])
        ah = big.tile([128, MAX_N * MAX_N], f32, name=f"adj{c}", tag=f"adj{c}")
        nc.scalar.copy(ah[:], ai[:])
        adj_h.append(ah)
        ft = big.tile([128, 3 * MAX_N], f32, name=f"feats{c}", tag=f"feats{c}")
        nc.sync.dma_start(ft[:], d["feats"][cs(c), :])
        feats.append(ft)

    wAc = wp.tile([109, GI], bf16, name="wAc", tag="wAc")
    nc.sync.dma_start(wAc[:], d["wAcomb"][:])
    whh = []
    for k, (ko, kk) in enumerate(KCH2):
        t = wp.tile([kk, GI], bf16, name=f"whh{k}", tag=f"whh{k}")
        nc.sync.dma_start(t[:], d["whh"][ko:ko + kk, :])
        whh.append(t)
    wpreI = wp.tile([19, HID], bf16, name="wpreI", tag="wpreI")
    nc.sync.dma_start(wpreI[:], d["wpreI"][:])
    wgm = []
    for k, (ko, kk) in enumerate(KCH2):
        t = wp.tile([kk, 602], bf16, name=f"wgm{k}", tag=f"wgm{k}")
        nc.sync.dma_start(t[:], d["wgm"][ko:ko + kk, :])
        wgm.append(t)
    wBc = wp.tile([109, 602], bf16, name="wBc", tag="wBc")
    nc.sync.dma_start(wBc[:], d["wBcomb"][:])
    wfc = []
    for k, sz in enumerate([128, 128, 97]):
        t = wp.tile([sz, OUT_W], bf16, name=f"wfc{k}", tag=f"wfc{k}")
        nc.sync.dma_start(t[:], d["wfc"][128 * k:128 * k + sz, :])
        wfc.append(t)
    wdf1 = wp.tile([33, 16], bf16, name="wdf1", tag="wdf1")
    nc.sync.dma_start(wdf1[:], d["wdf1"][:])
    wdf2 = wp.tile([33, 8], bf16, name="wdf2", tag="wdf2")
    nc.sync.dma_start(wdf2[:], d["wdf2"][:])

    # one-hot builds: fp16 copies on ACT, is_equal compares on DVE
    for c in range(CH):
        x = big.tile([128, MAX_N * 19], f32, name=f"X19_{c}", tag=f"X19_{c}")
        x3 = x[:].rearrange("p (v j) -> p v j", j=19)
        nc.vector.tensor_tensor(
            x3[:, :, 0:NT],
            tfs[c][:].unsqueeze(2).broadcast_to([128, MAX_N, NT]),
            cmpf[:, 0:NT].unsqueeze(1).broadcast_to([128, MAX_N, NT]),
            OP.is_equal,
        )
        nc.vector.tensor_tensor(
            x3[:, :, NT:19],
            pfs[c][:].unsqueeze(2).broadcast_to([128, MAX_N, PP]),
            cmpf[:, 0:PP].unsqueeze(1).broadcast_to([128, MAX_N, PP]),
            OP.is_equal,
        )
        X19.append(x)
        xb = big.tile([128, MAX_N * 19], f16, name=f"X19b_{c}", tag=f"X19b_{c}")
        nc.scalar.copy(xb[:], x[:])
        X19b.append(xb)

    # ---------------- combo tiles: one-hot rows prefilled per vertex ------
    # comboA_v: rows 0:19 one-hot(v), 64:109 h_in tail   (R matmul, K=109)
    # comboV_v: rows 0:19 one-hot(v), 64:109 h_v tail    (gate matmul, K=109)
    comboA, comboV = [], []
    for v in range(MAX_N):
        t = big.tile([109, BL], bf16, name=f"cA{v}", tag=f"cA{v}")
        # rows 19:64 must be zero (matching weight rows are zero); rows
        # 64:109 are written by the h_in-tail copies except at v=0.
        # Memsets must start at a 32-aligned partition, so zero 0:64 (the
        # one-hot copy overwrites rows 0:19 afterwards).
        nc.gpsimd.memset(t[0:64, :] if v > 0 else t[:, :], 0.0)
        comboA.append(t)
        if v < MAX_N - 1:
            t = big.tile([109, BL], bf16, name=f"cV{v}", tag=f"cV{v}")
            nc.gpsimd.memset(t[0:64, :], 0.0)
            comboV.append(t)
        else:
            comboV.append(None)
    for v2 in range(0, MAX_N, 2):
        ptx2 = PT.tile([128, 2 * BL], f16, name="pt", tag="pt")
        for dv in range(2):
            v = v2 + dv
            ptx = ptx2[:, dv * BL:(dv + 1) * BL]
            for c in range(CH):
                nc.tensor.transpose(
                    ptx[0:19, cs(c)], X19b[c][:, 19 * v:19 * v + 19],
                    ident_h[:]
                )
            nc.vector.tensor_copy(comboA[v][0:19, :], ptx[0:19, :])
            if v < MAX_N - 1:
                nc.scalar.copy(comboV[v][0:19, :], ptx[0:19, :])

    # gated message store, fp16, chunk-blocked [128, c, u, HP]; zeroed once
    # so the pad columns stay 0 (the far h_in updates read feature pairs
    # spanning col 301)
    G_all = big.tile([128, CH * 11 * HP], f16, name="G", tag="G")
    G4 = G_all[:].rearrange("p (c u j) -> p c u j", c=CH, u=11)
    nc.gpsimd.memset(G4[:, :, :, 301:302], 0.0)

    # ---------------- df scatter (independent of the scan; GPSIMD) --------
    dfT = big.tile([33, BL], bf16, name="dfT", tag="dfT")
    nc.vector.memset(dfT[:], 0.0)
    nc.vector.memset(dfT[32:33, :], 1.0)
    df_tiles = []
    for c in range(CH):
        df = big.tile([128, 27], f32, name=f"df{c}", tag=f"df{c}")
        nc.gpsimd.memset(df[:], 0.0)
        df3 = df[:].rearrange("p (q j) -> p q j", j=3)
        for v in range(MAX_N):
            f3 = feats[c][:, 3 * v:3 * v + 3].unsqueeze(1) \
                .broadcast_to([128, PP, 3])
            oh = X19[c][:, 19 * v + NT:19 * v + 19].unsqueeze(2) \
                .broadcast_to([128, PP, 3])
            s = p_df.tile([128, 27], f32, name="dfs", tag="dfs")
            s3 = s[:].rearrange("p (q j) -> p q j", j=3)
            nc.gpsimd.tensor_sub(s3, f3, df3)
            w = p_df.tile([128, 27], f32, name="dfw", tag="dfw")
            w3 = w[:].rearrange("p (q j) -> p q j", j=3)
            nc.gpsimd.tensor_mul(w3, oh, s3)
            nc.gpsimd.tensor_add(df3, df3, w3)
        df_tiles.append(df)

    # pair-duplicated adjacency for 2x-mode broadcast multiplies:
    # adjE[p, c, v, u, 0:2] = adj[p+128c, v, u]
    adjE = big.tile([128, CH * MAX_N * MAX_N * 2], f16, name="adjE", tag="adjE")
    adjE5 = adjE[:].rearrange("p (c v u d) -> p c v u d", c=CH, v=MAX_N, u=MAX_N)
    for c in range(CH):
        a3 = adj_h[c][:].rearrange("p (v u) -> p v u", v=MAX_N)
        nc.scalar.copy(
            adjE5[:, c], a3.unsqueeze(3).broadcast_to([128, MAX_N, MAX_N, 2]))

    # inn (x-side candidate pre-activation), fp16, chunk-blocked.
    # The matmuls for vertex v are emitted one step ahead inside the scan.
    inn_all = big.tile([128, CH * MAX_N * HP], f16, name="inn", tag="inn")
    inn4 = inn_all[:].rearrange("p (c v j) -> p c v j", c=CH, v=MAX_N)

    def emit_inn(v):
        for c in range(CH):
            pti = P1.tile([128, 512], f32, name="p1", tag="p1")
            mm(pti[:, 0:HID], comboA[v][0:19, cs(c)], wpreI[:],
               start=True, stop=True)
            nc.scalar.copy(inn4[:, c, v, 0:HID], pti[:, 0:HID])


    # h_in accumulators: slot w holds h_in for step w+1, incrementally
    # updated whenever a gated message lands (near: STT; far: TT pair)
    # no memset: slot w's first touch overwrites (near@0 for slot 0,
    # far@0 for slots 1..10)
    Hacc = big.tile([128, CH * 11 * HP], f16, name="Hacc", tag="Hacc")
    Ha4 = Hacc[:].rearrange("p (c w j) -> p c w j", c=CH, w=11)

    # shared mult scratch for far updates (DVE is in-order: safe to reuse)
    tmpF = big.tile([128, 2 * 10 * HP], f16, name="tmpF", tag="tmpF")
    _t4 = tmpF[:].rearrange("p (c w j) -> p c w j", c=2, w=10)
    tmpF4 = [_t4, _t4]

    # feature-major h_in / h buffers, double-buffered
    hiT = [[big.tile([128, BL], bf16, name=f"hiT{s}{k}", tag=f"hiT{s}{k}")
            for k in range(2)] for s in range(2)]
    for s in range(2):
        for k in range(2):
            nc.vector.memset(hiT[s][k][:], 0.0)
    hvT = [[big.tile([128, BL], bf16, name=f"hvT{s}{k}", tag=f"hvT{s}{k}")
            for k in range(2)] for s in range(2)]

    # sigmoid(r,z) planes [128, 2, CH, HP] and sigmoid-gate tile
    rzs = big.tile([128, 2 * CH * HP], f16, name="rzs", tag="rzs")
    rzs4 = rzs[:].rearrange("p (r c j) -> p r c j", r=2, c=CH)
    sgs = big.tile([128, CH * HP], f16, name="sgs", tag="sgs")
    sgs3 = sgs[:].rearrange("p (c j) -> p c j", c=CH)

    # FC tail lhsT: rows 0:45 h-tail, 64:72 Hd, 96 ones
    fcK2 = big.tile([97, BL], bf16, name="fcK2", tag="fcK2")
    nc.vector.memset(fcK2[:], 0.0)
    nc.vector.memset(fcK2[96:97, :], 1.0)

    # ---------------- the vertex scan ----------------
    # far-update closures deferred into the next step's DVE stream
    pending_far = []

    def emit_far(v, c):
        # Hacc[slots v+1..10] (+)= adj[., w+1, v] * G_v  (151 fp16 pairs;
        # col 301 is always 0 in G so the pad accumulates harmlessly).
        # v == 0 writes directly (first touch), later v accumulate.
        nw = 10 - v
        h2 = Ha4[:, c, v + 1:v + 1 + nw, 0:302].rearrange(
            "p w (e d) -> p w e d", d=2)
        g2 = G4[:, c, v, 0:302].rearrange("p (e d) -> p e d", d=2)
        a2 = adjE5[:, c, v + 2:v + 2 + nw, v, 0:2]
        dst = h2 if v == 0 else tmpF4[0][:, c % 2, 0:nw, 0:302].rearrange(
            "p w (e d) -> p w e d", d=2)
        nc.vector.tensor_tensor(
            dst,
            g2.unsqueeze(1).broadcast_to([128, nw, 151, 2]),
            a2.unsqueeze(2).broadcast_to([128, nw, 151, 2]),
            OP.mult)
        if v > 0:
            nc.vector.tensor_add(h2, h2, dst)

    for v in range(MAX_N):
        sb = v % 2
        ps = lambda p: slice(256 * p, 256 * (p + 1))

        if v > 0:
            hi3 = Ha4[:, :, v - 1, :]
        ptA = PT.tile([128, 2 * BL], f16, name="pt", tag="pt")
        ptB = PT.tile([128, 2 * BL], f16, name="pt", tag="pt")
        pts = [ptA[:, 0:BL], ptA[:, BL:2 * BL], ptB[:, 0:BL]]
        Rs = [None] * CH
        # phase 1 (per pair): h_in transposes -> copies -> R matmuls
        for p in range(2):
            if v > 0:
                for k, (ko, kk) in enumerate(KCH2):
                    for c in (2 * p, 2 * p + 1):
                        nc.tensor.transpose(pts[k][0:kk, cs(c)],
                                            hi3[:, c, ko:ko + kk], ident_h[:])
                    nc.vector.tensor_copy(hiT[sb][k][0:kk, ps(p)],
                                          pts[k][0:kk, ps(p)])
                for c in (2 * p, 2 * p + 1):
                    nc.tensor.transpose(pts[2][64:109, cs(c)],
                                        hi3[:, c, 256:HID], ident_h[:])
                nc.scalar.copy(comboA[v][64:109, ps(p)],
                               pts[2][64:109, ps(p)])
            for c in (2 * p, 2 * p + 1):
                R = PR.tile([128, 1024], f32, name="R", tag="R")
                cv = comboA[v][0:109, cs(c)]
                mm(R[:, 0:512], cv, wAc[:, 0:512], start=True, stop=False)
                mm(R[:, 512:903], cv, wAc[:, 512:903], start=True, stop=False)
                for k, (ko, kk) in enumerate(KCH2):
                    l = hiT[sb][k][0:kk, cs(c)]
                    last = k == 1
                    mm(R[:, 0:512], l, whh[k][:, 0:512],
                       start=False, stop=last)
                    mm(R[:, 512:903], l, whh[k][:, 512:903],
                       start=False, stop=last)
                Rs[c] = R
            if p == 0:
                # deferred far updates fill the DVE while the PE runs R
                for fn in pending_far:
                    fn()
                pending_far = []
                # x-side candidate for the next step rides the R phase
                if v == 0:
                    emit_inn(0)
                if v + 1 < MAX_N:
                    emit_inn(v + 1)

        # phase 2 (per pair): sigmoids + GRU elementwise
        tn = p16.tile([128, CH * HP], f16, name="tn", tag="tn")
        tn3 = tn[:].rearrange("p (c j) -> p c j", c=CH)
        tn2 = p16.tile([128, CH * HP], f16, name="tn2", tag="tn2")
        tn23 = tn2[:].rearrange("p (c j) -> p c j", c=CH)
        g = p16.tile([128, CH * HP], f16, name="g", tag="g")
        g3 = g[:].rearrange("p (c j) -> p c j", c=CH)
        t4 = p16.tile([128, CH * HP], f16, name="t4", tag="t4")
        t43 = t4[:].rearrange("p (c j) -> p c j", c=CH)
        hv = p16.tile([128, CH * HP], f16, name="hv", tag="hv")
        hv3 = hv[:].rearrange("p (c j) -> p c j", c=CH)
        if v > 0:
            t3 = p16.tile([128, CH * HP], f16, name="t3", tag="t3")
            t33 = t3[:].rearrange("p (c j) -> p c j", c=CH)
        for p in range(2):
            pc = slice(2 * p, 2 * p + 2)
            for c in (2 * p, 2 * p + 1):
                nc.scalar.activation(rzs4[:, 0:2, c, 0:HID],
                                     Rs[c][:, 0:602], AF.Sigmoid)
                nc.vector.tensor_mul(tn3[:, c, 0:HID], rzs4[:, 0, c, 0:HID],
                                     Rs[c][:, 602:903])
            nc.vector.tensor_add(tn23[:, pc, 0:HID], tn3[:, pc, 0:HID],
                                 inn4[:, pc, v, 0:HID])
            if p == 0:
                # keep the PE's HAM activity window alive through the
                # elementwise tail (junk transposes into a dead pt corner)
                nc.tensor.transpose(pts[2][0:64, 0:128],
                                    tn23[:, 0, 0:64], ident_h[:])
            nc.scalar.activation(g3[:, pc, 0:HID], tn23[:, pc, 0:HID],
                                 AF.Tanh)
            if v > 0:
                # hv = g + z*(h_in - g)
                nc.vector.tensor_sub(t33[:, pc, 0:HID], hi3[:, pc, 0:HID],
                                     g3[:, pc, 0:HID])
                nc.vector.tensor_mul(t43[:, pc, 0:HID], t33[:, pc, 0:HID],
                                     rzs4[:, 1, pc, 0:HID])
                nc.vector.tensor_add(hv3[:, pc, 0:HID], g3[:, pc, 0:HID],
                                     t43[:, pc, 0:HID])
            else:
                # h_in = 0: hv = g - z*g
                nc.vector.tensor_mul(t43[:, pc, 0:HID], g3[:, pc, 0:HID],
                                     rzs4[:, 1, pc, 0:HID])
                nc.vector.tensor_sub(hv3[:, pc, 0:HID], g3[:, pc, 0:HID],
                                     t43[:, pc, 0:HID])
            if p == 0:
                nc.tensor.transpose(pts[2][0:64, 128:256],
                                    t43[:, 0, 0:64], ident_h[:])

        # phase 3 (per pair): h_v transposes -> gates -> G_v -> near update
        ptC = PT.tile([128, 2 * BL], f16, name="pt", tag="pt")
        ptv = [ptB[:, BL:2 * BL], ptC[:, 0:BL], ptC[:, BL:2 * BL]]
        for p in range(2):
            for k, (ko, kk) in enumerate(KCH2):
                for c in (2 * p, 2 * p + 1):
                    nc.tensor.transpose(ptv[k][0:kk, cs(c)],
                                        hv3[:, c, ko:ko + kk], ident_h[:])
                nc.scalar.copy(hvT[sb][k][0:kk, ps(p)], ptv[k][0:kk, ps(p)])
            for c in (2 * p, 2 * p + 1):
                nc.tensor.transpose(ptv[2][64:109, cs(c)],
                                    hv3[:, c, 256:HID], ident_h[:])
            if v == MAX_N - 1:
                nc.scalar.copy(fcK2[0:45, ps(p)], ptv[2][64:109, ps(p)])
            else:
                nc.vector.tensor_copy(comboV[v][64:109, ps(p)],
                                      ptv[2][64:109, ps(p)])
            if v < MAX_N - 1:
                for c in (2 * p, 2 * p + 1):
                    Pg = P1.tile([128, 512], f32, name="p1", tag="p1")
                    Pm = P1.tile([128, 512], f32, name="p1", tag="p1")
                    cv = comboV[v][0:109, cs(c)]
                    mm(Pg[:, 0:HID], cv, wBc[:, 0:HID],
                       start=True, stop=False)
                    mm(Pm[:, 0:HID], cv, wBc[:, HID:602],
                       start=True, stop=False)
                    for k, (ko, kk) in enumerate(KCH2):
                        l = hvT[sb][k][0:kk, cs(c)]
                        last = k == 1
                        mm(Pg[:, 0:HID], l, wgm[k][:, 0:HID],
                           start=False, stop=last)
                        mm(Pm[:, 0:HID], l, wgm[k][:, HID:602],
                           start=False, stop=last)
                    nc.scalar.activation(sgs3[:, c, 0:HID], Pg[:, 0:HID],
                                         AF.Sigmoid)
                    nc.vector.tensor_mul(G4[:, c, v, 0:HID],
                                         sgs3[:, c, 0:HID], Pm[:, 0:HID])
                    # near update: Hacc[slot v] (+)= adj[v+1, v] * G_v
                    # (v == 0 is slot 0's first touch: overwrite)
                    asc = adj_h[c][:, MAX_N * (v + 1) + v:
                                   MAX_N * (v + 1) + v + 1]
                    if v == 0:
                        nc.vector.tensor_scalar_mul(
                            Ha4[:, c, v, 0:HID], G4[:, c, v, 0:HID], asc)
                    else:
                        nc.vector.scalar_tensor_tensor(
                            Ha4[:, c, v, 0:HID], G4[:, c, v, 0:HID], asc,
                            Ha4[:, c, v, 0:HID], op0=OP.mult, op1=OP.add)
        # far updates: pair-0 chunks now (fill tail), pair-1 deferred into
        # the next step's R phase
        if v < MAX_N - 2:
            emit_far(v, 0)
            emit_far(v, 1)
            pending_far = [
                (lambda vv, cc: (lambda: emit_far(vv, cc)))(v, c)
                for c in (2, 3)]
    for fn in pending_far:
        fn()
    pending_far = []

    # ---------------- df MLP ----------------
    for c in range(CH):
        pt = P1.tile([128, 512], f32, name="p1", tag="p1")
        nc.tensor.transpose(pt[0:27, 0:128], df_tiles[c][:], ident[:])
        nc.any.tensor_copy(dfT[0:27, cs(c)], pt[0:27, 0:128])
    pd1 = P1.tile([128, 512], f32, name="p1", tag="p1")
    mm(pd1[0:16, :], wdf1[:], dfT[:], start=True, stop=True)
    r1T = big.tile([33, BL], bf16, name="r1T", tag="r1T")
    nc.vector.memset(r1T[:], 0.0)
    nc.vector.memset(r1T[32:33, :], 1.0)
    nc.scalar.activation(r1T[0:16, :], pd1[0:16, :], AF.Relu)
    pd2 = P1.tile([128, 512], f32, name="p1", tag="p1")
    mm(pd2[0:8, :], wdf2[:], r1T[:], start=True, stop=True)
    nc.any.tensor_copy(fcK2[64:72, :], pd2[0:8, :])

    # ---------------- final fully-connected (mu | logvar) ----------------
    sb = (MAX_N - 1) % 2
    for c in range(CH):
        po = P1.tile([128, 512], f32, name="p1", tag="p1")
        mm(po[:, 0:OUT_W], hvT[sb][0][:, cs(c)], wfc[0][:],
           start=True, stop=False)
        mm(po[:, 0:OUT_W], hvT[sb][1][:, cs(c)], wfc[1][:],
           start=False, stop=False)
        mm(po[:, 0:OUT_W], fcK2[:, cs(c)], wfc[2][:], start=False, stop=True)
        ob = p_out.tile([128, OUT_W], f32, name="ob", tag="ob")
        nc.any.tensor_copy(ob[:], po[:, 0:OUT_W])
        nc.sync.dma_start(d_out[cs(c), :], ob[:])


def build_nc():
    nc = bacc.Bacc("TRN2", target_bir_lowering=False, debug=False,
                   num_devices=NCORES)
    d = {}
    for name, shape, dt in [
        ("types", [BL, MAX_N], i32),
        ("paths", [BL, MAX_N], i32),
        ("adj", [BL, MAX_N * MAX_N], i32),
        ("feats", [BL, 3 * MAX_N], f32),
        ("wpreA", [19, GI], bf16),
        ("wpreB", [19, 602], bf16),
        ("whh", [HID, GI], bf16),
        ("wpreI", [19, HID], bf16),
        ("wgm", [HID, 602], bf16),
        ("wBcomb", [109, 602], bf16),
        ("wAcomb", [109, GI], bf16),
        ("wfc", [353, OUT_W], bf16),
        ("wdf1", [33, 16], bf16),
        ("wdf2", [33, 8], bf16),
    ]:
        d[name] = nc.dram_tensor(name, shape, dt, kind="ExternalInput").ap()
    d_out = nc.dram_tensor("out", [BL, OUT_W], f32, kind="ExternalOutput").ap()
    with tile.TileContext(nc) as tc:
        with ExitStack() as ctx:
            _body(ctx, tc, nc, d, d_out)
    nc.compile()
    return nc


def prepack(inputs):
    ii = {k: np.asarray(v) for k, v in inputs.items()}
    W_ih, b_ih = ii["W_ih"].astype(np.float32), ii["b_ih"].astype(np.float32)
    Wg, bg = ii["Wg"].astype(np.float32), ii["bg"].astype(np.float32)
    Wm = ii["Wm"].astype(np.float32)
    b_hh = ii["b_hh"].astype(np.float32)
    # scan-side one-hot weights: gi r/z parts (+b_ih+b_hh) in [0:602];
    # [602:903] carries only b_hh's candidate part (inn itself is precomputed
    # separately via wpreI)
    wpreA = W_ih.T.copy()
    wpreA[:, 602:903] = 0.0
    wpreA[:NT, 0:602] += (b_ih + b_hh)[None, 0:602]
    wpreA[:NT, 602:903] += b_hh[None, 602:903]
    wpreI = W_ih.T[:, 602:903].copy()
    wpreI[:NT] += b_ih[None, 602:903]
    wpreB = np.zeros((19, 602), np.float32)
    wpreB[NT:19, 0:HID] = Wg[:, HID:HID + PP].T + bg[None, :]
    wpreB[NT:19, HID:602] = Wm[:, HID:HID + PP].T
    whh = ii["W_hh"].astype(np.float32).T.copy()
    wgm = np.zeros((HID, 602), np.float32)
    wgm[:, 0:HID] = Wg[:, 0:HID].T
    wgm[:, HID:602] = Wm[:, 0:HID].T
    wBcomb = np.zeros((109, 602), np.float32)
    wBcomb[0:19] = wpreB
    wBcomb[64:109] = wgm[256:301]
    wAcomb = np.zeros((109, GI), np.float32)
    wAcomb[0:19] = wpreA
    wAcomb[64:109] = whh[256:301]
    # FC lhsT rows: [0:256) = h dims 0:256 (two 128-chunks); tail chunk of 97
    # rows: 0:45 h-tail, 64:72 Hd, 96 biases (matches fcK2 on-device layout)
    wfcT1 = ii["W_fc1"].astype(np.float32).T   # [309, 56]
    wfcT2 = ii["W_fc2"].astype(np.float32).T
    wfc = np.zeros((353, OUT_W), np.float32)
    wfc[0:256, 0:56] = wfcT1[0:256]
    wfc[0:256, 56:112] = wfcT2[0:256]
    tail = np.zeros((97, OUT_W), np.float32)
    tail[0:45, 0:56] = wfcT1[256:301]
    tail[0:45, 56:112] = wfcT2[256:301]
    tail[64:72, 0:56] = wfcT1[301:309]
    tail[64:72, 56:112] = wfcT2[301:309]
    tail[96, 0:56] = ii["b_fc1"].astype(np.float32)
    tail[96, 56:112] = ii["b_fc2"].astype(np.float32)
    wfc[256:353] = tail
    wdf1 = np.zeros((33, 16), np.float32)
    wdf1[0:27] = ii["W_df1"].astype(np.float32).T
    wdf1[32] = ii["b_df1"].astype(np.float32)
    wdf2 = np.zeros((33, 8), np.float32)
    wdf2[0:16] = ii["W_df2"].astype(np.float32).T
    wdf2[32] = ii["b_df2"].astype(np.float32)
    import ml_dtypes
    out = dict(wpreA=wpreA, wpreB=wpreB, wpreI=wpreI, whh=whh, wgm=wgm,
               wBcomb=wBcomb, wAcomb=wAcomb,
               wfc=wfc, wdf1=wdf1, wdf2=wdf2)
    return {k: v.astype(ml_dtypes.bfloat16) for k, v in out.items()}


def shard_inputs(inputs):
    ii = {k: np.asarray(v) for k, v in inputs.items()}
    w = prepack(ii)
    maps = []
    for i in range(NCORES):
        sl = slice(i * BL, (i + 1) * BL)
        m = dict(
            types=np.ascontiguousarray(ii["types"][sl]).astype(np.int32),
            paths=np.ascontiguousarray(ii["paths"][sl]).astype(np.int32),
            adj=np.ascontiguousarray(
                ii["adj_raw"][sl].reshape(BL, MAX_N * MAX_N)).astype(np.int32),
            feats=np.ascontiguousarray(
                ii["feats"][sl].reshape(BL, 3 * MAX_N)).astype(np.float32),
            **w,
        )
        maps.append(m)
    return maps


def get_nc():
    if "nc" not in _CACHE:
        _CACHE["nc"] = build_nc()
    return _CACHE["nc"]


def kernel(**inputs):
    nc = get_nc()
    maps = shard_inputs(inputs)
    res = run_bass_kernel_spmd(nc, maps, list(range(NCORES)))
    out = np.concatenate([res.results[i]["out"] for i in range(NCORES)], axis=0)
    return np.ascontiguousarray(out.astype(np.float32))
